# revision 43
# baseline (speedup 1.0000x reference)
"""CenterLoss Trainium2 kernel (data-parallel over 8 NeuronCores).

loss = sum(clip(distmat * onehot(labels), 1e-12, 1e12)) / B with
distmat[i,c] = ||x_i - centers_c||^2. Only the (i, labels_i) entries survive
the mask; the B*(C-1) masked zeros contribute exactly 1e-12 each (added
analytically on host). The clip never binds on real entries (d_i ~ 4096), so

  sum_i d_i = sum_i ||x_i||^2 + sum_c n_c ||c_c||^2 - 2 sum_c <s_c, c_c>

with s = onehot(labels)^T @ x and n_c the per-class counts.

Device strategy (everything lands on the PE, engines only drain):

* Host sorts samples by label. Each core takes a contiguous 1024-sample run,
  so its labels span a <=128-class window (102 for the reference input) and
  the one-hot is only 128 wide. x ships as fp8e4 (rel err ~7e-4, measured,
  vs the 2e-2 gate), 2KB/partition chunks -> DMA at full 360B/ns.
* Centers ship once as cv = fp8(-2*c_c). The cross term drains as
  ps . cv = -2<s_c,c_c> (ps = onehot^T x from DoubleRow fp8 matmuls); the
  center-norm term n_c||c_c||^2 = sum Square((sqrt(n_c)/2)*cv) runs on the
  otherwise-idle Act engine mid-stream (per-partition activation scale).
* sum||x||^2 runs on the PE too: fp8 DoubleRow "Gram" matmuls of every
  128-dim block of x against itself, ALL accumulated into one [128,128]
  PSUM tile. diag(sum of block Grams) = per-dim sums of x^2; one 128-elem
  masked drain extracts it. No elementwise squares on DVE/Act/Pool at all.

Per-core output is a [128, 8] f32 block of raw partial columns
(col0 gram diag, col1/col3 split cross term, col2/col5 center-norm halves).
Host combine (f64): sum everything + B*(C-1)*1e-12, divide by B.
"""

from contextlib import ExitStack

import ml_dtypes
import numpy as np

import concourse.bacc as bacc
import concourse.tile as tile
from concourse import mybir
from concourse.bass_utils import run_bass_kernel_spmd

N_CORES = 8
B = 8192
D = 2048
C = 751
BS = B // N_CORES  # samples per core
P = 128
NT = BS // P       # sample tiles per core (8)
KDR = NT // 2      # fp8 DoubleRow pairs (4)
W = 128            # class window width per core
NCH = D // 512     # 512-col PSUM chunks for the class matmul (4)
NGB = D // P       # 128-dim Gram blocks (16)
OUTW = 8
FP8 = mybir.dt.float8e4
NP_FP8 = ml_dtypes.float8_e4m3
CLIP_LO = 1e-12

_NC = None


def build_nc():
    nc = bacc.Bacc("TRN2", target_bir_lowering=False)
    # xq[p, t, :] = fp8(x_sorted[shard, t*128 + p, :])
    xq = nc.dram_tensor("xq", [P, NT, D], FP8, kind="ExternalInput")
    # aux[p, 0:NT] = window-local label of sample t*128+p; aux[p, NT] = sqrt(n_p)/2
    aux = nc.dram_tensor("aux", [P, NT + 1], mybir.dt.float32, kind="ExternalInput")
    # cvirt[p, :] = fp8(-2 * centers[lo+p, :])
    cvirt = nc.dram_tensor("cvirt", [P, D], FP8, kind="ExternalInput")
    out = nc.dram_tensor("partial", [P, OUTW], mybir.dt.float32, kind="ExternalOutput")

    with tile.TileContext(nc) as tc, ExitStack() as ctx:
        perm = ctx.enter_context(tc.tile_pool(name="perm", bufs=1))
        scr = ctx.enter_context(tc.tile_pool(name="scr", bufs=2))
        psp = ctx.enter_context(tc.tile_pool(name="psp", bufs=1, space="PSUM"))

        # everything streams on the SP ring so HWDGE order matches need-order:
        # pair0 first (its 1456ns transfer overlaps aux's descriptor gen),
        # then aux (56ns, gates only the one-hots), remaining pairs, cvirt
        # mid-way. HWDGE gen is ~625ns per DMA, so chunks are [128, 2, 2048]
        # DR pairs to keep descriptor gen off the critical path.
        auxt = perm.tile([P, NT + 1], mybir.dt.float32)
        lab = auxt[:, 0:NT]
        sqn = auxt[:, NT : NT + 1]

        iota_i = perm.tile([P, W], mybir.dt.int32)
        nc.gpsimd.iota(iota_i[:], pattern=[[1, W]], base=0, channel_multiplier=0)
        iota_f = perm.tile([P, W], mybir.dt.float32)
        nc.vector.tensor_copy(out=iota_f[:], in_=iota_i[:])
        # ident[p, q] = (q == p), fp8-exact mask for the gram-diag drain
        pidx_i = perm.tile([P, 1], mybir.dt.int32)
        nc.gpsimd.iota(pidx_i[:], pattern=[[1, 1]], base=0, channel_multiplier=1)
        pidx_f = perm.tile([P, 1], mybir.dt.float32)
        nc.vector.tensor_copy(out=pidx_f[:], in_=pidx_i[:])
        ident = perm.tile([P, W], FP8)
        nc.vector.tensor_scalar(
            out=ident[:], in0=iota_f[:], scalar1=pidx_f[:], scalar2=None,
            op0=mybir.AluOpType.is_equal,
        )

        out_sb = perm.tile([P, OUTW], mybir.dt.float32)
        nc.vector.memset(out_sb[:], 0.0)

        # one-hots: oh[:, t, m] = (labf[p, t] == m)
        oh = perm.tile([P, NT, W], FP8)
        for t in range(NT):
            nc.vector.tensor_scalar(
                out=oh[:, t, :], in0=iota_f[:], scalar1=lab[:, t : t + 1],
                scalar2=None, op0=mybir.AluOpType.is_equal,
            )

        # x stream: one [128, 2, 2048] fp8 chunk per DR pair. cvirt ships in
        # two halves — the first mid-stream (feeds Act's first center-norm
        # op), the second last, landing just as k3's matmuls finish, so every
        # x pair arrives 728ns earlier than a mid-stream full-cv would allow.
        x8 = perm.tile([P, NT, D], FP8)
        cv = perm.tile([P, D], FP8)
        nc.sync.dma_start(out=x8[:, 0:2, :], in_=xq[:, 0:2, :])
        nc.sync.dma_start(out=auxt[:], in_=aux[:])
        nc.sync.dma_start(out=x8[:, 2:4, :], in_=xq[:, 2:4, :])
        nc.sync.dma_start(out=cv[:, 0:1024], in_=cvirt[:, 0:1024])
        nc.sync.dma_start(out=x8[:, 4:6, :], in_=xq[:, 4:6, :])
        # the final pair ships as two half-width chunks so k3's first two
        # class matmuls (and the first cross-drain half) start ~0.7us earlier
        nc.sync.dma_start(out=x8[:, 6:8, 0:1024], in_=xq[:, 6:8, 0:1024])
        nc.sync.dma_start(out=x8[:, 6:8, 1024:], in_=xq[:, 6:8, 1024:])
        nc.sync.dma_start(out=cv[:, 1024:], in_=cvirt[:, 1024:])

        # two class-PSUM tiles (feature halves) so each cross-drain half's
        # semaphore binds only to its own chunk's matmuls
        ps_a = psp.tile([P, D // 2], mybir.dt.float32, tag="clsa")
        ps_b = psp.tile([P, D // 2], mybir.dt.float32, tag="clsb")
        ps_gram = psp.tile([P, W], mybir.dt.float32, tag="gram")

        # p-state warm-up: the PE clock ramps with sustained use (0.65GHz cold,
        # 1.2GHz after ~100ns, 2.4GHz after 3us of continuous execution). Junk
        # matmuls from t~1us keep the ramp going so the real matmuls — which
        # chase the DMA stream — all price at full clock.
        junk = perm.tile([P, 2, W], FP8)
        nc.gpsimd.memset(junk[:], 0.0)
        ps_junk = psp.tile([P, W], mybir.dt.float32, tag="junk")
        NWARM = 88
        for i in range(NWARM):
            nc.tensor.matmul(
                out=ps_junk[:], lhsT=junk[:], rhs=junk[:],
                start=(i == 0), stop=(i == NWARM - 1),
                perf_mode=mybir.MatmulPerfMode.DoubleRow,
            )

        def cls_mm(k, n):
            pst = ps_a if n < 2 else ps_b
            nc.tensor.matmul(
                out=pst[:, 512 * (n % 2) : 512 * (n % 2 + 1)],
                lhsT=oh[:, 2 * k : 2 * k + 2, :],
                rhs=x8[:, 2 * k : 2 * k + 2, 512 * n : 512 * (n + 1)],
                start=(k == 0), stop=(k == KDR - 1),
                perf_mode=mybir.MatmulPerfMode.DoubleRow,
            )

        def gram_mm(k, g):
            blk = x8[:, 2 * k : 2 * k + 2, P * g : P * (g + 1)]
            nc.tensor.matmul(
                out=ps_gram[:], lhsT=blk, rhs=blk,
                start=(k == 0 and g == 0), stop=(k == KDR - 1 and g == NGB - 1),
                perf_mode=mybir.MatmulPerfMode.DoubleRow,
            )

        for k in range(KDR - 1):
            for n in range(NCH):
                cls_mm(k, n)
            for g in range(NGB):
                gram_mm(k, g)
        # k3 interleaves by half-pair arrival: everything reading cols
        # [0:1024] first, then the [1024:2048] half
        cls_mm(3, 0)
        cls_mm(3, 1)
        for g in range(NGB // 2):
            gram_mm(3, g)
        cls_mm(3, 2)
        cls_mm(3, 3)
        for g in range(NGB // 2, NGB):
            gram_mm(3, g)

        # center-norm term: sum Square((sqrt(n_p)/2) * cv) = n_p||c_p||^2,
        # on the otherwise-idle Act engine (one op per shipped cv half)
        cnsc = scr.tile([P, D], mybir.dt.float32, tag="cnsc")
        nc.scalar.activation(
            out=cnsc[:, 0:1024], in_=cv[:, 0:1024],
            func=mybir.ActivationFunctionType.Square,
            scale=sqn[:], accum_out=out_sb[:, 2:3],
        )
        nc.scalar.activation(
            out=cnsc[:, 1024:], in_=cv[:, 1024:],
            func=mybir.ActivationFunctionType.Square,
            scale=sqn[:], accum_out=out_sb[:, 5:6],
        )
        # cross-term drain: ps . cv = -2<s,c>, on DVE (the only engine that
        # can both read PSUM and do tensor*tensor on real HW); two halves
        # pipelined behind k3's half-pair matmuls
        csc = scr.tile([P, D], mybir.dt.bfloat16, tag="csc")
        nc.vector.scalar_tensor_tensor(
            out=csc[:, 0:1024], in0=ps_a[:],
            scalar=1.0, in1=cv[:, 0:1024],
            op0=mybir.AluOpType.mult, op1=mybir.AluOpType.mult,
            accum_out=out_sb[:, 1:2],
        )
        nc.vector.scalar_tensor_tensor(
            out=csc[:, 1024:], in0=ps_b[:],
            scalar=1.0, in1=cv[:, 1024:],
            op0=mybir.AluOpType.mult, op1=mybir.AluOpType.mult,
            accum_out=out_sb[:, 3:4],
        )
        # gram-diag drain: col0 += sum_q ps_gram[p, q] * ident[p, q].
        # Writes into csc's region so the scheduler keeps it AFTER the
        # cross-term drain on DVE (it becomes ready later but is shorter).
        nc.vector.scalar_tensor_tensor(
            out=csc[:, 0:W], in0=ps_gram[:], scalar=1.0, in1=ident[:],
            op0=mybir.AluOpType.mult, op1=mybir.AluOpType.mult,
            accum_out=out_sb[:, 0:1],
        )

        nc.sync.dma_start(out=out[:], in_=out_sb[:])
    nc.compile()
    return nc


def _pack_core(x_sh, lab_sh, centers, lo):
    """Per-core input arrays. x_sh/lab_sh already sorted by label."""
    span = int(lab_sh[-1]) - lo + 1
    assert span <= W, f"class window {span} exceeds {W}"
    xq = np.ascontiguousarray(
        x_sh.reshape(NT, P, D).transpose(1, 0, 2).astype(NP_FP8)
    )
    auxa = np.zeros((P, NT + 1), dtype=np.float32)
    auxa[:, :NT] = (lab_sh - lo).reshape(NT, P).T
    cnt = np.bincount(lab_sh - lo, minlength=W)[:W].astype(np.float64)
    auxa[:, NT] = (np.sqrt(cnt) / 2.0).astype(np.float32)
    cw = np.zeros((P, D), dtype=np.float64)
    hi = min(lo + W, C)
    cw[: hi - lo] = centers[lo:hi]
    cvirt = np.ascontiguousarray((cw * -2.0).astype(np.float32).astype(NP_FP8))
    return {"xq": xq, "aux": auxa, "cvirt": cvirt}


def make_in_maps(x, labels, centers):
    order = np.argsort(labels, kind="stable")
    xs = x[order]
    ls = labels[order].astype(np.int64)
    in_maps = []
    for c in range(N_CORES):
        sl = slice(c * BS, (c + 1) * BS)
        in_maps.append(_pack_core(xs[sl], ls[sl], centers, int(ls[sl.start])))
    return in_maps


def combine_partials(partials):
    total = 0.0
    for p in partials:
        total += float(np.sum(p.astype(np.float64)))
    total += float(B) * float(C - 1) * CLIP_LO
    return np.array(total / B, dtype=np.float32)


def kernel(**inputs) -> np.ndarray:
    global _NC
    x = np.ascontiguousarray(np.asarray(inputs["x"], dtype=np.float32))
    labels = np.asarray(inputs["labels"]).astype(np.int64)
    centers = np.ascontiguousarray(np.asarray(inputs["centers"], dtype=np.float32))
    assert x.shape == (B, D) and labels.shape == (B,) and centers.shape == (C, D)

    if _NC is None:
        _NC = build_nc()
    res = run_bass_kernel_spmd(
        _NC, make_in_maps(x, labels, centers), core_ids=list(range(N_CORES))
    )
    return combine_partials([r["partial"] for r in res.results])


# revision 47
# speedup vs baseline: 1.0498x; 1.0498x over previous
"""CenterLoss Trainium2 kernel (data-parallel over 8 NeuronCores).

loss = sum(clip(distmat * onehot(labels), 1e-12, 1e12)) / B with
distmat[i,c] = ||x_i - centers_c||^2. Only the (i, labels_i) entries survive
the mask; the B*(C-1) masked zeros contribute exactly 1e-12 each (added
analytically on host). The clip never binds on real entries (d_i ~ 4096), so

  sum_i d_i = sum_i ||x_i||^2 + sum_c n_c ||c_c||^2 - 2 sum_c <s_c, c_c>

with s = onehot(labels)^T @ x and n_c the per-class counts.

Device strategy (everything lands on the PE, engines only drain):

* Host sorts samples by label. Each core takes a contiguous 1024-sample run,
  so its labels span a <=128-class window (102 for the reference input) and
  the one-hot is only 128 wide. x ships as fp8e4 (rel err ~7e-4, measured,
  vs the 2e-2 gate), 2KB/partition chunks -> DMA at full 360B/ns.
* Centers ship once as cv = fp8(-2*c_c). The cross term drains as
  ps . cv = -2<s_c,c_c> (ps = onehot^T x from DoubleRow fp8 matmuls); the
  center-norm term n_c||c_c||^2 = sum Square((sqrt(n_c)/2)*cv) runs on the
  otherwise-idle Act engine mid-stream (per-partition activation scale).
* sum||x||^2 runs on the PE too: fp8 DoubleRow "Gram" matmuls of every
  128-dim block of x against itself, ALL accumulated into one [128,128]
  PSUM tile. diag(sum of block Grams) = per-dim sums of x^2; one 128-elem
  masked drain extracts it. No elementwise squares on DVE/Act/Pool at all.

Per-core output is a [128, 8] f32 block of raw partial columns
(col0 gram diag, col1/col3 split cross term, col2/col5 center-norm halves).
Host combine (f64): sum everything + B*(C-1)*1e-12, divide by B.
"""

from contextlib import ExitStack

import ml_dtypes
import numpy as np

import concourse.bacc as bacc
import concourse.tile as tile
from concourse import mybir
from concourse.bass_utils import run_bass_kernel_spmd

N_CORES = 8
B = 8192
D = 2048
C = 751
BS = B // N_CORES  # samples per core
P = 128
NT = BS // P       # sample tiles per core (8)
KDR = NT // 2      # fp8 DoubleRow pairs (4)
W = 128            # class window width per core
NCH = D // 512     # 512-col PSUM chunks for the class matmul (4)
NGB = D // P       # 128-dim Gram blocks (16)
OUTW = 8
FP8 = mybir.dt.float8e4
NP_FP8 = ml_dtypes.float8_e4m3
CLIP_LO = 1e-12

_NC = None


def build_nc():
    nc = bacc.Bacc("TRN2", target_bir_lowering=False)
    # xq[p, t, :] = fp8(x_sorted[shard, t*128 + p, :])
    xq = nc.dram_tensor("xq", [P, NT, D], FP8, kind="ExternalInput")
    # aux[p, 0:NT] = window-local label of sample t*128+p; aux[p, NT] = sqrt(n_p)/2
    aux = nc.dram_tensor("aux", [P, NT + 1], mybir.dt.float32, kind="ExternalInput")
    # cvirt[p, :] = fp8(-2 * centers[lo+p, :])
    cvirt = nc.dram_tensor("cvirt", [P, D], FP8, kind="ExternalInput")
    out = nc.dram_tensor("partial", [P, OUTW], mybir.dt.float32, kind="ExternalOutput")

    with tile.TileContext(nc) as tc, ExitStack() as ctx:
        perm = ctx.enter_context(tc.tile_pool(name="perm", bufs=1))
        scr = ctx.enter_context(tc.tile_pool(name="scr", bufs=2))
        psp = ctx.enter_context(tc.tile_pool(name="psp", bufs=1, space="PSUM"))

        # everything streams on the SP ring so HWDGE order matches need-order:
        # pair0 first (its 1456ns transfer overlaps aux's descriptor gen),
        # then aux (56ns, gates only the one-hots), remaining pairs, cvirt
        # mid-way. HWDGE gen is ~625ns per DMA, so chunks are [128, 2, 2048]
        # DR pairs to keep descriptor gen off the critical path.
        auxt = perm.tile([P, NT + 1], mybir.dt.float32)
        lab = auxt[:, 0:NT]
        sqn = auxt[:, NT : NT + 1]

        iota_i = perm.tile([P, W], mybir.dt.int32)
        nc.gpsimd.iota(iota_i[:], pattern=[[1, W]], base=0, channel_multiplier=0)
        iota_f = perm.tile([P, W], mybir.dt.float32)
        nc.vector.tensor_copy(out=iota_f[:], in_=iota_i[:])
        # ident[p, q] = (q == p), fp8-exact mask for the gram-diag drain
        pidx_i = perm.tile([P, 1], mybir.dt.int32)
        nc.gpsimd.iota(pidx_i[:], pattern=[[1, 1]], base=0, channel_multiplier=1)
        pidx_f = perm.tile([P, 1], mybir.dt.float32)
        nc.vector.tensor_copy(out=pidx_f[:], in_=pidx_i[:])
        ident = perm.tile([P, W], FP8)
        nc.vector.tensor_scalar(
            out=ident[:], in0=iota_f[:], scalar1=pidx_f[:], scalar2=None,
            op0=mybir.AluOpType.is_equal,
        )

        out_sb = perm.tile([P, OUTW], mybir.dt.float32)
        nc.vector.memset(out_sb[:], 0.0)

        # one-hots: oh[:, t, m] = (labf[p, t] == m)
        oh = perm.tile([P, NT, W], FP8)
        for t in range(NT):
            nc.vector.tensor_scalar(
                out=oh[:, t, :], in0=iota_f[:], scalar1=lab[:, t : t + 1],
                scalar2=None, op0=mybir.AluOpType.is_equal,
            )

        # x stream: one [128, 2, 2048] fp8 chunk per DR pair. cvirt ships in
        # two halves — the first mid-stream (feeds Act's first center-norm
        # op), the second last, landing just as k3's matmuls finish, so every
        # x pair arrives 728ns earlier than a mid-stream full-cv would allow.
        x8 = perm.tile([P, NT, D], FP8)
        cv = perm.tile([P, D], FP8)
        # all four pairs' cols [0:1024] ship as ONE big DMA: the whole ps_a
        # accumulation (and its cross-drain half) then completes mid-stream.
        # The cols [1024:2048] halves follow per-pair so k3's late matmuls
        # chase the smallest possible final chunk.
        nc.sync.dma_start(out=x8[:, :, 0:1024], in_=xq[:, :, 0:1024])
        nc.sync.dma_start(out=auxt[:], in_=aux[:])
        nc.sync.dma_start(out=cv[:, 0:1024], in_=cvirt[:, 0:1024])
        # pairs 0+1's b-halves merged (HWDGE gen budget), then ever-smaller
        # tail pieces, with cv quarters placed so no drain waits on cv
        nc.sync.dma_start(out=x8[:, 0:4, 1024:], in_=xq[:, 0:4, 1024:])
        nc.sync.dma_start(out=x8[:, 4:6, 1024:], in_=xq[:, 4:6, 1024:])
        nc.sync.dma_start(out=x8[:, 6:8, 1024:1536], in_=xq[:, 6:8, 1024:1536])
        nc.sync.dma_start(out=cv[:, 1024:1536], in_=cvirt[:, 1024:1536])
        nc.sync.dma_start(out=x8[:, 6:8, 1536:], in_=xq[:, 6:8, 1536:])
        nc.sync.dma_start(out=cv[:, 1536:], in_=cvirt[:, 1536:])

        # two class-PSUM tiles (feature halves) so each cross-drain half's
        # semaphore binds only to its own chunk's matmuls
        ps_a = psp.tile([P, D // 2], mybir.dt.float32, tag="clsa")
        ps_b2 = psp.tile([P, 512], mybir.dt.float32, tag="clsb2")
        ps_b3 = psp.tile([P, 512], mybir.dt.float32, tag="clsb3")
        ps_gram = psp.tile([P, W], mybir.dt.float32, tag="gram")

        # p-state warm-up: the PE clock ramps with sustained use (0.65GHz cold,
        # 1.2GHz after ~100ns, 2.4GHz after 3us of continuous execution). Junk
        # matmuls from t~1us keep the ramp going so the real matmuls — which
        # chase the DMA stream — all price at full clock.
        junk = perm.tile([P, 2, W], FP8)
        nc.gpsimd.memset(junk[:], 0.0)
        ps_junk = psp.tile([P, W], mybir.dt.float32, tag="junk")
        NWARM = 88
        for i in range(NWARM):
            nc.tensor.matmul(
                out=ps_junk[:], lhsT=junk[:], rhs=junk[:],
                start=(i == 0), stop=(i == NWARM - 1),
                perf_mode=mybir.MatmulPerfMode.DoubleRow,
            )

        def cls_mm(k, n):
            pst = ps_a if n < 2 else (ps_b2 if n == 2 else ps_b3)
            nc.tensor.matmul(
                out=pst[:, 512 * n : 512 * (n + 1)] if n < 2 else pst[:, 0:512],
                lhsT=oh[:, 2 * k : 2 * k + 2, :],
                rhs=x8[:, 2 * k : 2 * k + 2, 512 * n : 512 * (n + 1)],
                start=(k == 0), stop=(k == KDR - 1),
                perf_mode=mybir.MatmulPerfMode.DoubleRow,
            )

        def gram_mm(k, g):
            blk = x8[:, 2 * k : 2 * k + 2, P * g : P * (g + 1)]
            nc.tensor.matmul(
                out=ps_gram[:], lhsT=blk, rhs=blk,
                start=(k == 0 and g == 0), stop=(k == KDR - 1 and g == NGB - 1),
                perf_mode=mybir.MatmulPerfMode.DoubleRow,
            )

        for k in range(KDR):
            cls_mm(k, 0)
            cls_mm(k, 1)
        for k in range(KDR):
            for g in range(NGB // 2):
                gram_mm(k, g)
        for k in range(KDR - 1):
            cls_mm(k, 2)
            cls_mm(k, 3)
            for g in range(NGB // 2, NGB):
                gram_mm(k, g)
        # k3's b-half work ordered by quarter-chunk arrival
        cls_mm(3, 2)
        for g in range(8, 12):
            gram_mm(3, g)
        cls_mm(3, 3)
        for g in range(12, NGB):
            gram_mm(3, g)

        # center-norm term: sum Square((sqrt(n_p)/2) * cv) = n_p||c_p||^2,
        # on the otherwise-idle Act engine (one op per shipped cv half)
        cnsc = scr.tile([P, D], mybir.dt.float32, tag="cnsc")
        nc.scalar.activation(
            out=cnsc[:, 0:1024], in_=cv[:, 0:1024],
            func=mybir.ActivationFunctionType.Square,
            scale=sqn[:], accum_out=out_sb[:, 2:3],
        )
        nc.scalar.activation(
            out=cnsc[:, 1024:], in_=cv[:, 1024:],
            func=mybir.ActivationFunctionType.Square,
            scale=sqn[:], accum_out=out_sb[:, 5:6],
        )
        # cross-term drain: ps . cv = -2<s,c>, on DVE (the only engine that
        # can both read PSUM and do tensor*tensor on real HW); two halves
        # pipelined behind k3's half-pair matmuls
        csc = scr.tile([P, D], mybir.dt.bfloat16, tag="csc")
        nc.vector.scalar_tensor_tensor(
            out=csc[:, 0:1024], in0=ps_a[:],
            scalar=1.0, in1=cv[:, 0:1024],
            op0=mybir.AluOpType.mult, op1=mybir.AluOpType.mult,
            accum_out=out_sb[:, 1:2],
        )
        nc.vector.scalar_tensor_tensor(
            out=csc[:, 1024:1536], in0=ps_b2[:],
            scalar=1.0, in1=cv[:, 1024:1536],
            op0=mybir.AluOpType.mult, op1=mybir.AluOpType.mult,
            accum_out=out_sb[:, 3:4],
        )
        nc.vector.scalar_tensor_tensor(
            out=csc[:, 1536:], in0=ps_b3[:],
            scalar=1.0, in1=cv[:, 1536:],
            op0=mybir.AluOpType.mult, op1=mybir.AluOpType.mult,
            accum_out=out_sb[:, 4:5],
        )
        # gram-diag drain: col0 += sum_q ps_gram[p, q] * ident[p, q].
        # Writes into csc's region so the scheduler keeps it AFTER the
        # cross-term drain on DVE (it becomes ready later but is shorter).
        nc.vector.scalar_tensor_tensor(
            out=csc[:, 0:W], in0=ps_gram[:], scalar=1.0, in1=ident[:],
            op0=mybir.AluOpType.mult, op1=mybir.AluOpType.mult,
            accum_out=out_sb[:, 0:1],
        )

        nc.sync.dma_start(out=out[:], in_=out_sb[:])
    nc.compile()
    return nc


def _pack_core(x_sh, lab_sh, centers, lo):
    """Per-core input arrays. x_sh/lab_sh already sorted by label."""
    span = int(lab_sh[-1]) - lo + 1
    assert span <= W, f"class window {span} exceeds {W}"
    xq = np.ascontiguousarray(
        x_sh.reshape(NT, P, D).transpose(1, 0, 2).astype(NP_FP8)
    )
    auxa = np.zeros((P, NT + 1), dtype=np.float32)
    auxa[:, :NT] = (lab_sh - lo).reshape(NT, P).T
    cnt = np.bincount(lab_sh - lo, minlength=W)[:W].astype(np.float64)
    auxa[:, NT] = (np.sqrt(cnt) / 2.0).astype(np.float32)
    cw = np.zeros((P, D), dtype=np.float64)
    hi = min(lo + W, C)
    cw[: hi - lo] = centers[lo:hi]
    cvirt = np.ascontiguousarray((cw * -2.0).astype(np.float32).astype(NP_FP8))
    return {"xq": xq, "aux": auxa, "cvirt": cvirt}


def make_in_maps(x, labels, centers):
    order = np.argsort(labels, kind="stable")
    xs = x[order]
    ls = labels[order].astype(np.int64)
    in_maps = []
    for c in range(N_CORES):
        sl = slice(c * BS, (c + 1) * BS)
        in_maps.append(_pack_core(xs[sl], ls[sl], centers, int(ls[sl.start])))
    return in_maps


def combine_partials(partials):
    total = 0.0
    for p in partials:
        total += float(np.sum(p.astype(np.float64)))
    total += float(B) * float(C - 1) * CLIP_LO
    return np.array(total / B, dtype=np.float32)


def kernel(**inputs) -> np.ndarray:
    global _NC
    x = np.ascontiguousarray(np.asarray(inputs["x"], dtype=np.float32))
    labels = np.asarray(inputs["labels"]).astype(np.int64)
    centers = np.ascontiguousarray(np.asarray(inputs["centers"], dtype=np.float32))
    assert x.shape == (B, D) and labels.shape == (B,) and centers.shape == (C, D)

    if _NC is None:
        _NC = build_nc()
    res = run_bass_kernel_spmd(
        _NC, make_in_maps(x, labels, centers), core_ids=list(range(N_CORES))
    )
    return combine_partials([r["partial"] for r in res.results])
